# revision 1
# baseline (speedup 1.0000x reference)
"""CrissCrossAttention Trainium2 kernel.

Full inputs in, full output out. Data-parallel over batch across 8 cores
(B=16 -> 2 images per core). Per image (H=W=128, C=256, D=32):

  - x is uploaded pre-cast to bf16 twice: `xbf` (exact x, consumed via a
    DMA-XBAR transposed load into channel-on-partition layout XT) and
    `xres` (x + gamma*bv, the residual source, consumed in natural layout).
  - qT/kT [32, pix] from XT with Wq/Wk stationary (fp32 PSUM, bias add on
    drain, cast bf16).
  - column branch (per image column w): JIT v2 tile [h,256] from XT slices,
    energies transposed eT[k,h] = Kw Qw^T (K=32 matmul), four w's packed
    into one PSUM bank, the -120*I diagonal mask accumulated with a single
    wide I^T @ (-120*[I I I I]) matmul, one exp per bank, aggregation
    U_h[h, 257] = exp^T.T @ [v2|1] (ones column gives the softmax
    denominator for free). U_h tiles stream to a DRAM scratch with an
    (h,w)-swapping scatter so they come back in natural order.
  - row branch (per image row h): same machinery with v1/h-slices; the
    U_h merge is an accumulating I.T @ uh matmul into the same PSUM as
    U_w, and the epilogue computes
    out = xres + gamma * (U_h + U_w) / (S_h + S_w).

All matmuls bf16 with fp32 PSUM accumulation; residual uses bf16(x+g*bv).
"""

import os
import sys

import numpy as np

try:
    import concourse  # noqa: F401
except ImportError:
    for p in ("/root/.axon_site/_ro/trn_rl_repo", "/opt/trn_rl_repo"):
        if os.path.isdir(p):
            sys.path.insert(0, p)
            break

import ml_dtypes

import concourse.bass as bass  # noqa: F401
import concourse.tile as tile
from concourse import bacc, mybir
from concourse.bass_utils import run_bass_kernel_spmd

BF16 = mybir.dt.bfloat16
F32 = mybir.dt.float32
AF = mybir.ActivationFunctionType

B, H, W, C, D = 16, 128, 128, 256, 32
NCORES = 8
BPC = B // NCORES  # images per core
HWPIX = H * W
GAMMA = 0.05
NEGBIG = -120.0  # diagonal mask offset; exp(e-120) underflows to 0
CU = C + 1  # v tiles carry a ones column -> softmax denominator


def build_program():
    nc = bacc.Bacc(
        "TRN2",
        target_bir_lowering=False,
        debug=False,
        num_devices=NCORES,
    )

    xbf = nc.dram_tensor("xbf", [BPC, HWPIX, C], BF16, kind="ExternalInput").ap()
    xres = nc.dram_tensor("xres", [BPC, HWPIX, C], BF16, kind="ExternalInput").ap()
    wq_d = nc.dram_tensor("wq_b", [2, 128, D], BF16, kind="ExternalInput").ap()
    wk_d = nc.dram_tensor("wk_b", [2, 128, D], BF16, kind="ExternalInput").ap()
    wv_d = nc.dram_tensor("wv_b", [2, 128, C], BF16, kind="ExternalInput").ap()
    bq_d = nc.dram_tensor("bq_f", [D, 1], F32, kind="ExternalInput").ap()
    bk_d = nc.dram_tensor("bk_f", [D, 1], F32, kind="ExternalInput").ap()
    eye_d = nc.dram_tensor("eye_b", [128, 128], BF16, kind="ExternalInput").ap()
    negi4_d = nc.dram_tensor("negi4_b", [128, 512], BF16, kind="ExternalInput").ap()
    uh_d = nc.dram_tensor("uh_scratch", [BPC, HWPIX, CU], BF16, kind="Internal").ap()
    out_d = nc.dram_tensor("out", [BPC, HWPIX, C], F32, kind="ExternalOutput").ap()

    HGRP = 8  # rows staged per DMA

    with tile.TileContext(nc) as tc:
        with (
            tc.tile_pool(name="const", bufs=1) as constp,
            tc.tile_pool(name="xt", bufs=1) as xtp,
            tc.tile_pool(name="qkt", bufs=1) as qktp,
            tc.tile_pool(name="vtile", bufs=5) as vp,
            tc.tile_pool(name="etile", bufs=4) as ep,
            tc.tile_pool(name="ustage", bufs=3) as usp,
            tc.tile_pool(name="uload", bufs=3) as ulp,
            tc.tile_pool(name="xr", bufs=3) as xrp,
            tc.tile_pool(name="ost", bufs=3) as osp,
            tc.tile_pool(name="rwork", bufs=6) as rp,
            tc.tile_pool(name="psv", bufs=2, space="PSUM") as psv,
            tc.tile_pool(name="pse", bufs=2, space="PSUM") as pse,
            tc.tile_pool(name="psu", bufs=2, space="PSUM") as psu,
        ):
            wq_sb = constp.tile([128, 2, D], BF16)
            wk_sb = constp.tile([128, 2, D], BF16)
            wv_sb = constp.tile([128, 2, C], BF16)
            bq_sb = constp.tile([D, 1], F32)
            bk_sb = constp.tile([D, 1], F32)
            eye_sb = constp.tile([128, 128], BF16)
            negi4_sb = constp.tile([128, 512], BF16)
            nc.sync.dma_start(wq_sb[:], wq_d.rearrange("c p d -> p c d"))
            nc.sync.dma_start(wk_sb[:], wk_d.rearrange("c p d -> p c d"))
            nc.sync.dma_start(wv_sb[:], wv_d.rearrange("c p d -> p c d"))
            nc.sync.dma_start(bq_sb[:], bq_d)
            nc.sync.dma_start(bk_sb[:], bk_d)
            nc.sync.dma_start(eye_sb[:], eye_d)
            nc.sync.dma_start(negi4_sb[:], negi4_d)

            def v_pair(xtv, p0, p1, strided):
                """Project v for two pixel-slices into one PSUM bank pair,
                drain to a [128, 2, CU] bf16 tile with ones column."""
                pv = psv.tile([128, 2, C], F32, tag="pv")
                for j, p in enumerate((p0, p1)):
                    for cc in range(2):
                        lhs = xtv[:, cc, :, p] if strided else xtv[:, cc, p, :]
                        nc.tensor.matmul(
                            pv[:, j, :], lhs, wv_sb[:, cc, :],
                            start=(cc == 0), stop=(cc == 1),
                        )
                vt = vp.tile([128, 2, CU], BF16, tag="vt")
                if strided:
                    nc.scalar.activation(vt[:, :, :C], pv[:], AF.Copy)
                else:
                    nc.vector.tensor_copy(vt[:, :, :C], pv[:])
                nc.vector.memset(vt[:, :, C], 1.0)
                return vt

            for bi in range(BPC):
                # ---- transposed x: XT[cpart, chunk, pix] (pix h-major) ----
                xt = xtp.tile([128, 2, HWPIX], BF16)
                QT4 = HWPIX // 4
                for qq in range(4):
                    psl = slice(qq * QT4, (qq + 1) * QT4)
                    for cc in range(2):
                        # all transposes on one queue: the XBAR is a single
                        # shared unit, concurrent transposes corrupt data
                        nc.sync.dma_start(
                            xt[:, cc, psl],
                            xbf[bi, psl, cc * 128 : (cc + 1) * 128],
                            transpose=True,
                        )
                xtv = xt.rearrange("p c (h w) -> p c h w", h=H)

                # ---- qT/kT projections ----
                qt = qktp.tile([D, HWPIX], BF16, tag="qt")
                kt = qktp.tile([D, HWPIX], BF16, tag="kt")
                for pc in range(HWPIX // 512):
                    sl = slice(pc * 512, (pc + 1) * 512)
                    pq = psu.tile([32, 2, 512], F32, tag="pu")
                    nc.tensor.matmul(pq[:, 0, :], wq_sb[:, 0, :], xt[:, 0, sl], start=True, stop=False)
                    nc.tensor.matmul(pq[:, 0, :], wq_sb[:, 1, :], xt[:, 1, sl], start=False, stop=True)
                    nc.tensor.matmul(pq[:, 1, :], wk_sb[:, 0, :], xt[:, 0, sl], start=True, stop=False)
                    nc.tensor.matmul(pq[:, 1, :], wk_sb[:, 1, :], xt[:, 1, sl], start=False, stop=True)
                    nc.vector.tensor_scalar_add(qt[:, sl], pq[:, 0, :], bq_sb[:])
                    nc.vector.tensor_scalar_add(kt[:, sl], pq[:, 1, :], bk_sb[:])
                qtv = qt.rearrange("p (h w) -> p h w", h=H)
                ktv = kt.rearrange("p (h w) -> p h w", h=H)

                # ---- phase A: column attention, U_h -> DRAM scratch ----
                uh_v = uh_d[bi].rearrange("(h w) c -> h w c", h=H)
                for wg in range(W // HGRP):
                    ust = usp.tile([128, HGRP, CU], BF16)
                    for q4 in range(HGRP // 4):  # 4 w's per energy bank
                        wq4 = wg * HGRP + q4 * 4
                        pe4 = pse.tile([128, 4, 128], F32, tag="pe")
                        for i in range(4):
                            # start=True clears has_written for the WHOLE
                            # bank, so only the first matmul may set it.
                            nc.tensor.matmul(
                                pe4[:, i, :], ktv[:, :, wq4 + i], qtv[:, :, wq4 + i],
                                start=(i == 0), stop=False, skip_group_check=True,
                            )
                        nc.tensor.matmul(
                            pe4.rearrange("p a b -> p (a b)"), eye_sb[:], negi4_sb[:],
                            start=False, stop=True, skip_group_check=True,
                        )
                        ex4 = ep.tile([128, 4, 128], BF16, tag="ex")
                        nc.scalar.activation(ex4[:], pe4[:], AF.Exp)

                        vta = v_pair(xtv, wq4 + 0, wq4 + 1, strided=True)
                        vtb = v_pair(xtv, wq4 + 2, wq4 + 3, strided=True)
                        pu_a = psu.tile([128, 2, 512], F32, tag="pu")
                        nc.tensor.matmul(pu_a[:, 0, :CU], ex4[:, 0, :], vta[:, 0, :], start=True, stop=True)
                        nc.tensor.matmul(pu_a[:, 1, :CU], ex4[:, 1, :], vta[:, 1, :], start=True, stop=True)
                        pu_b = psu.tile([128, 2, 512], F32, tag="pu")
                        nc.tensor.matmul(pu_b[:, 0, :CU], ex4[:, 2, :], vtb[:, 0, :], start=True, stop=True)
                        nc.tensor.matmul(pu_b[:, 1, :CU], ex4[:, 3, :], vtb[:, 1, :], start=True, stop=True)
                        wi = q4 * 4
                        nc.vector.tensor_copy(ust[:, wi : wi + 2, :], pu_a[:, :, :CU])
                        nc.vector.tensor_copy(ust[:, wi + 2 : wi + 4, :], pu_b[:, :, :CU])
                    nc.sync.dma_start(uh_v[:, wg * HGRP : (wg + 1) * HGRP, :], ust[:])

                # ---- phase B: row attention + merge + epilogue ----
                uh_w = uh_d[bi].rearrange("(h w) c -> w h c", h=H)
                xr_w = xres[bi].rearrange("(h w) c -> w h c", h=H)
                out_w = out_d[bi].rearrange("(h w) c -> w h c", h=H)
                for hg in range(H // HGRP):
                    hsl = slice(hg * HGRP, (hg + 1) * HGRP)
                    ul = ulp.tile([128, HGRP, CU], BF16)
                    nc.sync.dma_start(ul[:], uh_w[:, hsl, :])
                    xrt = xrp.tile([128, HGRP, C], BF16)
                    nc.sync.dma_start(xrt[:], xr_w[:, hsl, :])
                    ost = osp.tile([128, HGRP, C], F32)
                    for q4 in range(HGRP // 4):
                        hq4 = hg * HGRP + q4 * 4
                        pe4 = pse.tile([128, 4, 128], F32, tag="pe")
                        for i in range(4):
                            nc.tensor.matmul(
                                pe4[:, i, :], ktv[:, hq4 + i, :], qtv[:, hq4 + i, :],
                                start=(i == 0), stop=(i == 3), skip_group_check=True,
                            )
                        ex4 = ep.tile([128, 4, 128], BF16, tag="ex")
                        nc.scalar.activation(ex4[:], pe4[:], AF.Exp)

                        vta = v_pair(xtv, hq4 + 0, hq4 + 1, strided=False)
                        vtb = v_pair(xtv, hq4 + 2, hq4 + 3, strided=False)
                        for pair in range(2):
                            pu2 = psu.tile([128, 2, 512], F32, tag="pu")
                            vt = (vta, vtb)[pair]
                            for j in range(2):
                                i = pair * 2 + j
                                hi = q4 * 4 + i
                                nc.tensor.matmul(
                                    pu2[:, j, :CU], ex4[:, i, :], vt[:, j, :],
                                    start=True, stop=False, skip_group_check=True,
                                )
                                nc.tensor.matmul(
                                    pu2[:, j, :CU], eye_sb[:], ul[:, hi, :],
                                    start=False, stop=True, skip_group_check=True,
                                )
                                gs = rp.tile([128, 1], F32, tag="gs")
                                nc.vector.reciprocal(gs, pu2[:, j, C : C + 1])
                                gs2 = rp.tile([128, 1], F32, tag="gs2")
                                nc.vector.tensor_scalar_mul(gs2, gs, GAMMA)
                                r2 = rp.tile([128, C], F32, tag="r2")
                                nc.scalar.activation(r2, pu2[:, j, :C], AF.Copy, scale=gs2[:])
                                nc.gpsimd.tensor_add(ost[:, hi, :], r2, xrt[:, hi, :])
                    nc.sync.dma_start(out_w[:, hsl, :], ost[:])

    nc.compile()
    return nc


_NC_CACHE = None


def _get_nc():
    global _NC_CACHE
    if _NC_CACHE is None:
        _NC_CACHE = build_program()
    return _NC_CACHE


def make_in_maps(x, wq, bq, wk, bk, wv, bv):
    bf = ml_dtypes.bfloat16
    x = np.asarray(x, np.float32)
    xres_full = (x + GAMMA * np.asarray(bv, np.float32)).astype(bf)
    xbf_full = x.astype(bf)
    eye = np.eye(128, dtype=bf)
    negi4 = np.tile((NEGBIG * np.eye(128, dtype=np.float32)).astype(bf), (1, 4))

    in_maps = []
    for ci in range(NCORES):
        sl = slice(ci * BPC, (ci + 1) * BPC)
        in_maps.append(
            {
                "xbf": xbf_full[sl].reshape(BPC, HWPIX, C),
                "xres": xres_full[sl].reshape(BPC, HWPIX, C),
                "wq_b": np.asarray(wq, np.float32).astype(bf).reshape(2, 128, D),
                "wk_b": np.asarray(wk, np.float32).astype(bf).reshape(2, 128, D),
                "wv_b": np.asarray(wv, np.float32).astype(bf).reshape(2, 128, C),
                "bq_f": np.asarray(bq, np.float32).reshape(D, 1),
                "bk_f": np.asarray(bk, np.float32).reshape(D, 1),
                "eye_b": eye,
                "negi4_b": negi4,
            }
        )
    return in_maps


def kernel(x, wq, bq, wk, bk, wv, bv):
    in_maps = make_in_maps(x, wq, bq, wk, bk, wv, bv)
    nc = _get_nc()
    res = run_bass_kernel_spmd(nc, in_maps, core_ids=list(range(NCORES)))
    outs = [res.results[ci]["out"].reshape(BPC, H, W, C) for ci in range(NCORES)]
    return np.concatenate(outs, axis=0).astype(np.float32)



# revision 16
# speedup vs baseline: 1.3520x; 1.3520x over previous
"""CrissCrossAttention Trainium2 kernel (v3).

Full inputs in, full output out. Data-parallel over batch across 8 cores
(B=16 -> 2 images per core). Per image (H=W=128, C=256, D=32):

  - x is uploaded HOST-pre-transposed as fp8e4m3 `xt8` [128, 2, HWPIX]
    (channel-on-partition; no XBAR DMA transposes) plus `xres`
    (bf16, natural layout, = x + gamma*bv, the residual source).
  - q/k projection: fp8 DoubleRow matmuls with [wq|wk] stationary
    (contraction C=256 in one instruction), fp32 PSUM, bias add on
    drain, cast bf16 -> qt/kt [32, pix].
  - v projection (both branches): fp8 DoubleRow with the x pixel-slice
    stationary ([128,2,128] lhsT) and wv moving ([128,2,256]) -- one
    matmul per 128-pixel slice. gamma is folded into the PSUM->SBUF
    drain (scale=GAMMA), so the epilogue needs no extra scaling.
  - column branch (per image column w): energies eT[k,h] = Kw Qw^T
    (K=32 matmul, bf16), four w's packed into one PSUM bank, -120*I
    diagonal mask via one wide I^T @ (-120*[I I I I]) matmul, one exp
    per bank, aggregation U_h[h, 257] = exp^T.T @ [gamma*v|1] (ones
    column gives the softmax denominator). U_h tiles stream to a DRAM
    scratch with an (h,w)-swapping scatter.
  - row branch (per image row h): same machinery with natural slices;
    U_h merge is an accumulating I.T @ uh matmul into the same PSUM as
    U_w (aggs first, then merges, so the eye stationary loads once per
    group). Epilogue: batched reciprocal of the ones-column sums, then
    one fused scalar_tensor_tensor per row:
        out = (U * (1/S)) + xres        (U already carries gamma)
    alternating vector/gpsimd. Output stored bf16, upcast on host.

All attention matmuls bf16 with fp32 PSUM accumulation.
"""

import os
import sys

import numpy as np

try:
    import concourse  # noqa: F401
except ImportError:
    for p in ("/root/.axon_site/_ro/trn_rl_repo", "/opt/trn_rl_repo"):
        if os.path.isdir(p):
            sys.path.insert(0, p)
            break

import ml_dtypes

import concourse.bass as bass  # noqa: F401
import concourse.tile as tile
from concourse import bacc, mybir
from concourse.bass_utils import run_bass_kernel_spmd

BF16 = mybir.dt.bfloat16
F32 = mybir.dt.float32
FP8 = mybir.dt.float8e4
AF = mybir.ActivationFunctionType
ALU = mybir.AluOpType
DR = mybir.MatmulPerfMode.DoubleRow

B, H, W, C, D = 16, 128, 128, 256, 32
NCORES = 8
BPC = B // NCORES  # images per core
HWPIX = H * W
GAMMA = 0.05
NEGBIG = -120.0  # diagonal mask offset; exp(e-120) underflows to 0
CU = C + 1  # v tiles carry a ones column -> softmax denominator


def build_program():
    nc = bacc.Bacc(
        "TRN2",
        target_bir_lowering=False,
        debug=False,
        num_devices=NCORES,
    )

    xt8_d = nc.dram_tensor("xt8", [BPC, 128, 2, HWPIX], FP8, kind="ExternalInput").ap()
    xres = nc.dram_tensor("xres", [BPC, HWPIX, C], BF16, kind="ExternalInput").ap()
    wqk_d = nc.dram_tensor("wqk8", [128, 2, 2 * D], FP8, kind="ExternalInput").ap()
    wv_d = nc.dram_tensor("wv8", [128, 2, C], FP8, kind="ExternalInput").ap()
    bq_d = nc.dram_tensor("bq_f", [D, 1], F32, kind="ExternalInput").ap()
    bk_d = nc.dram_tensor("bk_f", [D, 1], F32, kind="ExternalInput").ap()
    eye_d = nc.dram_tensor("eye_b", [128, 128], BF16, kind="ExternalInput").ap()
    mask4_d = nc.dram_tensor("mask4_b", [128, 512], BF16, kind="ExternalInput").ap()
    uh_d = nc.dram_tensor("uh_scratch", [BPC, HWPIX, CU], BF16, kind="Internal").ap()
    out_d = nc.dram_tensor("out", [BPC, HWPIX, C], BF16, kind="ExternalOutput").ap()

    HGRP = 8  # rows staged per DMA

    with tile.TileContext(nc) as tc:
        with (
            tc.tile_pool(name="const", bufs=1) as constp,
            tc.tile_pool(name="xt", bufs=2) as xtp,
            tc.tile_pool(name="qkt", bufs=1) as qktp,
            tc.tile_pool(name="vtile", bufs=5) as vp,
            tc.tile_pool(name="etile", bufs=4) as ep,
            tc.tile_pool(name="ustage", bufs=3) as usp,
            tc.tile_pool(name="uload", bufs=3) as ulp,
            tc.tile_pool(name="xr", bufs=3) as xrp,
            tc.tile_pool(name="ost", bufs=3) as osp,
            tc.tile_pool(name="rwork", bufs=6) as rp,
            tc.tile_pool(name="psv", bufs=2, space="PSUM") as psv,
            tc.tile_pool(name="pse", bufs=2, space="PSUM") as pse,
            tc.tile_pool(name="psu", bufs=2, space="PSUM") as psu,
        ):
            wqk_sb = constp.tile([128, 2, 2 * D], FP8)
            wv_sb = constp.tile([128, 2, C], FP8)
            bq_sb = constp.tile([D, 1], F32)
            bk_sb = constp.tile([D, 1], F32)
            eye_sb = constp.tile([128, 128], BF16)
            mask4_sb = constp.tile([128, 512], BF16)
            nc.sync.dma_start(wqk_sb[:], wqk_d)
            nc.sync.dma_start(wv_sb[:], wv_d)
            nc.sync.dma_start(bq_sb[:], bq_d)
            nc.sync.dma_start(bk_sb[:], bk_d)
            nc.sync.dma_start(eye_sb[:], eye_d)
            nc.sync.dma_start(mask4_sb[:], mask4_d)

            def v_pair(xtv, p0, p1, strided, on_scalar):
                """Project gamma*v for two pixel-slices (DoubleRow fp8),
                drain to a [128, 2, CU] bf16 tile with ones column."""
                pv = psv.tile([128, 2, C], F32, tag="pv")
                for j, p in enumerate((p0, p1)):
                    lhs = xtv[:, :, :, p] if strided else xtv[:, :, p, :]
                    nc.tensor.matmul(
                        pv[:, j, :], lhs, wv_sb[:], start=True, stop=True,
                        perf_mode=DR,
                    )
                vt = vp.tile([128, 2, CU], BF16, tag="vt")
                if on_scalar:
                    nc.scalar.activation(vt[:, :, :C], pv[:], AF.Copy, scale=GAMMA)
                else:
                    nc.vector.tensor_scalar_mul(vt[:, :, :C], pv[:], GAMMA)
                nc.gpsimd.memset(vt[:, :, C], 1.0)
                return vt

            for bi in range(BPC):
                # ---- transposed x: XT[cpart, chunk, pix] (pix h-major) ----
                xt = xtp.tile([128, 2, HWPIX], FP8)
                QT8 = HWPIX // 8
                for qq in range(8):
                    psl = slice(qq * QT8, (qq + 1) * QT8)
                    nc.sync.dma_start(xt[:, :, psl], xt8_d[bi, :, :, psl])
                xtv = xt.rearrange("p c (h w) -> p c h w", h=H)

                # ---- qT/kT projections (fp8 DoubleRow, wqk stationary) ----
                # qt/kt are replicated to all four 32-partition groups so the
                # energy matmuls can run 4-way row-tiled (K=32 packing).
                qt = qktp.tile([D, HWPIX], BF16, tag="qt")
                kt = qktp.tile([D, HWPIX], BF16, tag="kt")
                for pc in range(HWPIX // 512):
                    sl = slice(pc * 512, (pc + 1) * 512)
                    pq = psu.tile([128, 2, 512], F32, tag="pu")
                    for half in range(2):
                        hs = slice(pc * 512 + half * 256, pc * 512 + (half + 1) * 256)
                        nc.tensor.matmul(
                            pq[0 : 2 * D, 0, half * 256 : (half + 1) * 256],
                            wqk_sb[:], xt[:, :, hs],
                            start=(half == 0), stop=(half == 1),
                            perf_mode=DR, skip_group_check=True,
                        )
                    nc.scalar.add(qt[0:D, sl], pq[0:D, 0, :], bq_sb[:])
                    nc.vector.tensor_scalar_add(kt[0:D, sl], pq[D : 2 * D, 0, :], bk_sb[:])
                qtv = qt.rearrange("p (h w) -> p h w", h=H)
                ktv = kt.rearrange("p (h w) -> p h w", h=H)

                # ---- phase A: column attention, U_h -> DRAM scratch ----
                uh_v = uh_d[bi].rearrange("(h w) c -> h w c", h=H)
                for wg in range(W // HGRP):
                    ust = usp.tile([128, HGRP, CU], BF16)
                    for q4 in range(HGRP // 4):  # 4 w's per energy bank
                        wq4 = wg * HGRP + q4 * 4
                        pe4 = pse.tile([128, 4, 128], F32, tag="pe")
                        for i in range(4):
                            # 4-way row-tiled: each w's K=32 contraction in its
                            # own 32-row group so the matmuls run concurrently
                            # and their weight loads pull ahead.
                            nc.tensor.matmul(
                                pe4[:, i, :],
                                ktv[0:D, :, wq4 + i],
                                qtv[0:D, :, wq4 + i],
                                start=(i == 0), stop=(i == 3), skip_group_check=True,
                            )
                        ex4 = ep.tile([128, 4, 128], BF16, tag="ex")
                        nc.scalar.activation(ex4[:], pe4[:], AF.Exp)
                        # diagonal mask: exp(e-120) == exp(e) * (1-I)
                        exm = ep.tile([128, 4, 128], BF16, tag="exm")
                        nc.gpsimd.tensor_mul(
                            exm.rearrange("p a b -> p (a b)"),
                            ex4.rearrange("p a b -> p (a b)"), mask4_sb[:],
                        )

                        vta = v_pair(xtv, wq4 + 0, wq4 + 1, True, on_scalar=True)
                        vtb = v_pair(xtv, wq4 + 2, wq4 + 3, True, on_scalar=False)
                        pu_a = psu.tile([128, 2, 512], F32, tag="pu")
                        nc.tensor.matmul(pu_a[:, 0, :CU], exm[:, 0, :], vta[:, 0, :], start=True, stop=True)
                        nc.tensor.matmul(pu_a[:, 1, :CU], exm[:, 1, :], vta[:, 1, :], start=True, stop=True)
                        pu_b = psu.tile([128, 2, 512], F32, tag="pu")
                        nc.tensor.matmul(pu_b[:, 0, :CU], exm[:, 2, :], vtb[:, 0, :], start=True, stop=True)
                        nc.tensor.matmul(pu_b[:, 1, :CU], exm[:, 3, :], vtb[:, 1, :], start=True, stop=True)
                        wi = q4 * 4
                        nc.vector.tensor_copy(ust[:, wi : wi + 2, :], pu_a[:, :, :CU])
                        nc.scalar.activation(ust[:, wi + 2 : wi + 4, :], pu_b[:, :, :CU], AF.Copy)
                    nc.sync.dma_start(uh_v[:, wg * HGRP : (wg + 1) * HGRP, :], ust[:])

                # ---- phase B: row attention + merge + epilogue ----
                uh_w = uh_d[bi].rearrange("(h w) c -> w h c", h=H)
                xr_w = xres[bi].rearrange("(h w) c -> w h c", h=H)
                out_w = out_d[bi].rearrange("(h w) c -> w h c", h=H)
                for hg in range(H // HGRP):
                    hsl = slice(hg * HGRP, (hg + 1) * HGRP)
                    ul = ulp.tile([128, HGRP, CU], BF16)
                    nc.sync.dma_start(ul[:], uh_w[:, hsl, :])
                    xrt = xrp.tile([128, HGRP, C], BF16)
                    nc.sync.dma_start(xrt[:], xr_w[:, hsl, :])
                    ost = osp.tile([128, HGRP, C], BF16)
                    for q4 in range(HGRP // 4):
                        hq4 = hg * HGRP + q4 * 4
                        pe4 = pse.tile([128, 4, 128], F32, tag="pe")
                        for i in range(4):
                            nc.tensor.matmul(
                                pe4[:, i, :],
                                ktv[0:D, hq4 + i, :],
                                qtv[0:D, hq4 + i, :],
                                start=(i == 0), stop=(i == 3), skip_group_check=True,
                            )
                        ex4 = ep.tile([128, 4, 128], BF16, tag="ex")
                        nc.scalar.activation(ex4[:], pe4[:], AF.Exp)

                        vta = v_pair(xtv, hq4 + 0, hq4 + 1, False, on_scalar=False)
                        vtb = v_pair(xtv, hq4 + 2, hq4 + 3, False, on_scalar=True)
                        pu2 = [
                            psu.tile([128, 2, 512], F32, tag="pu", name=f"pu2{p}")
                            for p in range(2)
                        ]
                        # aggs first, then merges: the eye stationary for the
                        # merges loads once per group instead of per matmul.
                        for pair in range(2):
                            vt = (vta, vtb)[pair]
                            for j in range(2):
                                nc.tensor.matmul(
                                    pu2[pair][:, j, :CU], ex4[:, pair * 2 + j, :], vt[:, j, :],
                                    start=True, stop=False, skip_group_check=True,
                                )
                        for pair in range(2):
                            for j in range(2):
                                hi = q4 * 4 + pair * 2 + j
                                nc.tensor.matmul(
                                    pu2[pair][:, j, :CU], eye_sb[:], ul[:, hi, :],
                                    start=False, stop=True, skip_group_check=True,
                                )
                        for pair in range(2):
                            gs = rp.tile([128, 2, 1], F32, tag="gs")
                            nc.vector.reciprocal(gs, pu2[pair][:, :, C : C + 1])
                            for j in range(2):
                                hi = q4 * 4 + pair * 2 + j
                                if j == 0:
                                    # gpsimd can't read PSUM: scale on vector,
                                    # residual add on gpsimd (SBUF only)
                                    r2 = rp.tile([128, C], BF16, tag="r2")
                                    nc.vector.tensor_scalar_mul(
                                        r2, pu2[pair][:, j, :C], gs[:, j, :]
                                    )
                                    nc.gpsimd.tensor_add(
                                        ost[:, hi, :], r2, xrt[:, hi, :]
                                    )
                                else:
                                    nc.vector.scalar_tensor_tensor(
                                        ost[:, hi, :], pu2[pair][:, j, :C], gs[:, j, :],
                                        xrt[:, hi, :], op0=ALU.mult, op1=ALU.add,
                                    )
                    nc.sync.dma_start(out_w[:, hsl, :], ost[:])

    nc.compile()
    return nc


_NC_CACHE = None


def _get_nc():
    global _NC_CACHE
    if _NC_CACHE is None:
        _NC_CACHE = build_program()
    return _NC_CACHE


def make_in_maps(x, wq, bq, wk, bk, wv, bv):
    bf = ml_dtypes.bfloat16
    f8 = ml_dtypes.float8_e4m3
    x = np.asarray(x, np.float32)
    xres_full = (x + GAMMA * np.asarray(bv, np.float32)).astype(bf)
    # host-side transpose to channel-on-partition fp8: [b, 128, 2, HWPIX]
    xt8_full = np.ascontiguousarray(
        x.reshape(B, HWPIX, 2, 128).transpose(0, 3, 2, 1)
    ).astype(f8)
    wqk = np.concatenate(
        [np.asarray(wq, np.float32), np.asarray(wk, np.float32)], axis=1
    )  # [C, 64]
    wqk8 = np.ascontiguousarray(wqk.reshape(2, 128, 2 * D).transpose(1, 0, 2)).astype(f8)
    wv8 = np.ascontiguousarray(
        np.asarray(wv, np.float32).reshape(2, 128, C).transpose(1, 0, 2)
    ).astype(f8)
    eye = np.eye(128, dtype=bf)
    mask4 = np.tile((1.0 - np.eye(128, dtype=np.float32)).astype(bf), (1, 4))

    in_maps = []
    for ci in range(NCORES):
        sl = slice(ci * BPC, (ci + 1) * BPC)
        in_maps.append(
            {
                "xt8": xt8_full[sl],
                "xres": xres_full[sl].reshape(BPC, HWPIX, C),
                "wqk8": wqk8,
                "wv8": wv8,
                "bq_f": np.asarray(bq, np.float32).reshape(D, 1),
                "bk_f": np.asarray(bk, np.float32).reshape(D, 1),
                "eye_b": eye,
                "mask4_b": mask4,
            }
        )
    return in_maps


def kernel(x, wq, bq, wk, bk, wv, bv):
    in_maps = make_in_maps(x, wq, bq, wk, bk, wv, bv)
    nc = _get_nc()
    res = run_bass_kernel_spmd(nc, in_maps, core_ids=list(range(NCORES)))
    outs = [
        np.asarray(res.results[ci]["out"]).astype(np.float32).reshape(BPC, H, W, C)
        for ci in range(NCORES)
    ]
    return np.concatenate(outs, axis=0)


# revision 17
# speedup vs baseline: 1.3991x; 1.0348x over previous
"""CrissCrossAttention Trainium2 kernel (v3).

Full inputs in, full output out. Data-parallel over batch across 8 cores
(B=16 -> 2 images per core). Per image (H=W=128, C=256, D=32):

  - x is uploaded HOST-pre-transposed as fp8e4m3 `xt8` [128, 2, HWPIX]
    (channel-on-partition; no XBAR DMA transposes) plus `xres`
    (bf16, natural layout, = x + gamma*bv, the residual source).
  - q/k projection: fp8 DoubleRow matmuls with [wq|wk] stationary
    (contraction C=256 in one instruction), fp32 PSUM, bias add on
    drain, cast bf16 -> qt/kt [32, pix].
  - v projection (both branches): fp8 DoubleRow with the x pixel-slice
    stationary ([128,2,128] lhsT) and wv moving ([128,2,256]) -- one
    matmul per 128-pixel slice. gamma is folded into the PSUM->SBUF
    drain (scale=GAMMA), so the epilogue needs no extra scaling.
  - column branch (per image column w): energies eT[k,h] = Kw Qw^T
    (K=32 matmul, bf16), four w's packed into one PSUM bank, -120*I
    diagonal mask via one wide I^T @ (-120*[I I I I]) matmul, one exp
    per bank, aggregation U_h[h, 257] = exp^T.T @ [gamma*v|1] (ones
    column gives the softmax denominator). U_h tiles stream to a DRAM
    scratch with an (h,w)-swapping scatter.
  - row branch (per image row h): same machinery with natural slices;
    U_h merge is an accumulating I.T @ uh matmul into the same PSUM as
    U_w (aggs first, then merges, so the eye stationary loads once per
    group). Epilogue: batched reciprocal of the ones-column sums, then
    one fused scalar_tensor_tensor per row:
        out = (U * (1/S)) + xres        (U already carries gamma)
    alternating vector/gpsimd. Output stored bf16, upcast on host.

All attention matmuls bf16 with fp32 PSUM accumulation.
"""

import os
import sys

import numpy as np

try:
    import concourse  # noqa: F401
except ImportError:
    for p in ("/root/.axon_site/_ro/trn_rl_repo", "/opt/trn_rl_repo"):
        if os.path.isdir(p):
            sys.path.insert(0, p)
            break

import ml_dtypes

import concourse.bass as bass  # noqa: F401
import concourse.tile as tile
from concourse import bacc, mybir
from concourse.bass_utils import run_bass_kernel_spmd

BF16 = mybir.dt.bfloat16
F32 = mybir.dt.float32
FP8 = mybir.dt.float8e4
AF = mybir.ActivationFunctionType
ALU = mybir.AluOpType
DR = mybir.MatmulPerfMode.DoubleRow

B, H, W, C, D = 16, 128, 128, 256, 32
NCORES = 8
BPC = B // NCORES  # images per core
HWPIX = H * W
GAMMA = 0.05
NEGBIG = -120.0  # diagonal mask offset; exp(e-120) underflows to 0
CU = C + 1  # v tiles carry a ones column -> softmax denominator


def build_program():
    nc = bacc.Bacc(
        "TRN2",
        target_bir_lowering=False,
        debug=False,
        num_devices=NCORES,
    )

    xt8_d = nc.dram_tensor("xt8", [BPC, 128, 2, HWPIX], FP8, kind="ExternalInput").ap()
    xres = nc.dram_tensor("xres", [BPC, HWPIX, C], BF16, kind="ExternalInput").ap()
    wqk_d = nc.dram_tensor("wqk8", [128, 2, 2 * D], FP8, kind="ExternalInput").ap()
    wv_d = nc.dram_tensor("wv8", [128, 2, C], FP8, kind="ExternalInput").ap()
    bq_d = nc.dram_tensor("bq_f", [D, 1], F32, kind="ExternalInput").ap()
    bk_d = nc.dram_tensor("bk_f", [D, 1], F32, kind="ExternalInput").ap()
    eye_d = nc.dram_tensor("eye_b", [128, 128], BF16, kind="ExternalInput").ap()
    mask4_d = nc.dram_tensor("mask4_b", [128, 512], BF16, kind="ExternalInput").ap()
    uh_d = nc.dram_tensor("uh_scratch", [BPC, HWPIX, CU], BF16, kind="Internal").ap()
    out_d = nc.dram_tensor("out", [BPC, HWPIX, C], BF16, kind="ExternalOutput").ap()

    HGRP = 8  # rows staged per DMA

    with tile.TileContext(nc) as tc:
        with (
            tc.tile_pool(name="const", bufs=1) as constp,
            tc.tile_pool(name="xt", bufs=2) as xtp,
            tc.tile_pool(name="qkt", bufs=1) as qktp,
            tc.tile_pool(name="vtile", bufs=5) as vp,
            tc.tile_pool(name="etile", bufs=4) as ep,
            tc.tile_pool(name="ustage", bufs=3) as usp,
            tc.tile_pool(name="uload", bufs=3) as ulp,
            tc.tile_pool(name="xr", bufs=3) as xrp,
            tc.tile_pool(name="ost", bufs=3) as osp,
            tc.tile_pool(name="rwork", bufs=6) as rp,
            tc.tile_pool(name="psv", bufs=2, space="PSUM") as psv,
            tc.tile_pool(name="pse", bufs=2, space="PSUM") as pse,
            tc.tile_pool(name="psu", bufs=2, space="PSUM") as psu,
        ):
            wqk_sb = constp.tile([128, 2, 2 * D], FP8)
            wv_sb = constp.tile([128, 2, C], FP8)
            bq_sb = constp.tile([D, 1], F32)
            bk_sb = constp.tile([D, 1], F32)
            eye_sb = constp.tile([128, 128], BF16)
            mask4_sb = constp.tile([128, 512], BF16)
            nc.sync.dma_start(wqk_sb[:], wqk_d)
            nc.sync.dma_start(wv_sb[:], wv_d)
            nc.sync.dma_start(bq_sb[:], bq_d)
            nc.sync.dma_start(bk_sb[:], bk_d)
            nc.sync.dma_start(eye_sb[:], eye_d)
            nc.sync.dma_start(mask4_sb[:], mask4_d)

            def v_pair(xtv, p0, p1, strided, on_scalar):
                """Project gamma*v for two pixel-slices (DoubleRow fp8),
                drain to a [128, 2, CU] bf16 tile with ones column."""
                pv = psv.tile([128, 2, C], F32, tag="pv")
                for j, p in enumerate((p0, p1)):
                    lhs = xtv[:, :, :, p] if strided else xtv[:, :, p, :]
                    nc.tensor.matmul(
                        pv[:, j, :], lhs, wv_sb[:], start=True, stop=True,
                        perf_mode=DR,
                    )
                vt = vp.tile([128, 2, CU], BF16, tag="vt")
                if on_scalar:
                    nc.scalar.activation(vt[:, :, :C], pv[:], AF.Copy, scale=GAMMA)
                else:
                    nc.vector.tensor_scalar_mul(vt[:, :, :C], pv[:], GAMMA)
                nc.gpsimd.memset(vt[:, :, C], 1.0)
                return vt

            for bi in range(BPC):
                # ---- transposed x: XT[cpart, chunk, pix] (pix h-major) ----
                xt = xtp.tile([128, 2, HWPIX], FP8)
                QT8 = HWPIX // 8
                for qq in range(8):
                    psl = slice(qq * QT8, (qq + 1) * QT8)
                    nc.sync.dma_start(xt[:, :, psl], xt8_d[bi, :, :, psl])
                xtv = xt.rearrange("p c (h w) -> p c h w", h=H)

                # ---- qT/kT projections (fp8 DoubleRow, wqk stationary) ----
                # qt/kt are replicated to all four 32-partition groups so the
                # energy matmuls can run 4-way row-tiled (K=32 packing).
                qt = qktp.tile([D, HWPIX], BF16, tag="qt")
                kt = qktp.tile([D, HWPIX], BF16, tag="kt")
                for pc in range(HWPIX // 512):
                    sl = slice(pc * 512, (pc + 1) * 512)
                    pq = psu.tile([128, 2, 512], F32, tag="pu")
                    for half in range(2):
                        hs = slice(pc * 512 + half * 256, pc * 512 + (half + 1) * 256)
                        nc.tensor.matmul(
                            pq[0 : 2 * D, 0, half * 256 : (half + 1) * 256],
                            wqk_sb[:], xt[:, :, hs],
                            start=(half == 0), stop=(half == 1),
                            perf_mode=DR, skip_group_check=True,
                        )
                    nc.scalar.add(qt[0:D, sl], pq[0:D, 0, :], bq_sb[:])
                    nc.vector.tensor_scalar_add(kt[0:D, sl], pq[D : 2 * D, 0, :], bk_sb[:])
                qtv = qt.rearrange("p (h w) -> p h w", h=H)
                ktv = kt.rearrange("p (h w) -> p h w", h=H)

                # ---- phase A: column attention, U_h -> DRAM scratch ----
                uh_v = uh_d[bi].rearrange("(h w) c -> h w c", h=H)
                for wg in range(W // HGRP):
                    ust = usp.tile([128, HGRP, CU], BF16)
                    for q4 in range(HGRP // 4):  # 4 w's per energy bank
                        wq4 = wg * HGRP + q4 * 4
                        pe4 = pse.tile([128, 4, 128], F32, tag="pe")
                        for i in range(4):
                            # 4-way row-tiled: each w's K=32 contraction in its
                            # own 32-row group so the matmuls run concurrently
                            # and their weight loads pull ahead.
                            nc.tensor.matmul(
                                pe4[:, i, :],
                                ktv[0:D, :, wq4 + i],
                                qtv[0:D, :, wq4 + i],
                                start=(i == 0), stop=(i == 3), skip_group_check=True,
                            )
                        ex4 = ep.tile([128, 4, 128], BF16, tag="ex")
                        nc.scalar.activation(ex4[:], pe4[:], AF.Exp)
                        # diagonal mask: exp(e-120) == exp(e) * (1-I)
                        exm = ep.tile([128, 4, 128], BF16, tag="exm")
                        nc.gpsimd.tensor_mul(
                            exm.rearrange("p a b -> p (a b)"),
                            ex4.rearrange("p a b -> p (a b)"), mask4_sb[:],
                        )

                        vta = v_pair(xtv, wq4 + 0, wq4 + 1, True, on_scalar=True)
                        vtb = v_pair(xtv, wq4 + 2, wq4 + 3, True, on_scalar=False)
                        pu_a = psu.tile([128, 2, 512], F32, tag="pu")
                        nc.tensor.matmul(pu_a[:, 0, :CU], exm[:, 0, :], vta[:, 0, :], start=True, stop=True)
                        nc.tensor.matmul(pu_a[:, 1, :CU], exm[:, 1, :], vta[:, 1, :], start=True, stop=True)
                        pu_b = psu.tile([128, 2, 512], F32, tag="pu")
                        nc.tensor.matmul(pu_b[:, 0, :CU], exm[:, 2, :], vtb[:, 0, :], start=True, stop=True)
                        nc.tensor.matmul(pu_b[:, 1, :CU], exm[:, 3, :], vtb[:, 1, :], start=True, stop=True)
                        wi = q4 * 4
                        nc.vector.tensor_copy(ust[:, wi : wi + 2, :], pu_a[:, :, :CU])
                        nc.scalar.activation(ust[:, wi + 2 : wi + 4, :], pu_b[:, :, :CU], AF.Copy)
                    nc.sync.dma_start(uh_v[:, wg * HGRP : (wg + 1) * HGRP, :], ust[:])

                # ---- phase B: row attention + merge + epilogue ----
                uh_w = uh_d[bi].rearrange("(h w) c -> w h c", h=H)
                xr_w = xres[bi].rearrange("(h w) c -> w h c", h=H)
                out_w = out_d[bi].rearrange("(h w) c -> w h c", h=H)
                for hg in range(H // HGRP):
                    hsl = slice(hg * HGRP, (hg + 1) * HGRP)
                    ul = ulp.tile([128, HGRP, CU], BF16)
                    nc.sync.dma_start(ul[:], uh_w[:, hsl, :])
                    xrt = xrp.tile([128, HGRP, C], BF16)
                    nc.sync.dma_start(xrt[:], xr_w[:, hsl, :])
                    ost = osp.tile([128, HGRP, C], BF16)
                    for q4 in range(HGRP // 4):
                        hq4 = hg * HGRP + q4 * 4
                        pe4 = pse.tile([128, 4, 128], F32, tag="pe")
                        for i in range(4):
                            nc.tensor.matmul(
                                pe4[:, i, :],
                                ktv[0:D, hq4 + i, :],
                                qtv[0:D, hq4 + i, :],
                                start=(i == 0), stop=(i == 3), skip_group_check=True,
                            )
                        ex4 = ep.tile([128, 4, 128], BF16, tag="ex")
                        nc.scalar.activation(ex4[:], pe4[:], AF.Exp)

                        vta = v_pair(xtv, hq4 + 0, hq4 + 1, False, on_scalar=False)
                        vtb = v_pair(xtv, hq4 + 2, hq4 + 3, False, on_scalar=True)
                        pu2 = [
                            psu.tile([128, 2, 512], F32, tag="pu", name=f"pu2{p}")
                            for p in range(2)
                        ]
                        # aggs first, then merges: the eye stationary for the
                        # merges loads once per group instead of per matmul.
                        for pair in range(2):
                            vt = (vta, vtb)[pair]
                            for j in range(2):
                                nc.tensor.matmul(
                                    pu2[pair][:, j, :CU], ex4[:, pair * 2 + j, :], vt[:, j, :],
                                    start=True, stop=False, skip_group_check=True,
                                )
                        gss = []
                        for pair in range(2):
                            for j in range(2):
                                hi = q4 * 4 + pair * 2 + j
                                nc.tensor.matmul(
                                    pu2[pair][:, j, :CU], eye_sb[:], ul[:, hi, :],
                                    start=False, stop=True, skip_group_check=True,
                                )
                            # reciprocal right after this pair's merges so the
                            # PSUM-freeing scale ops aren't stuck behind queued
                            # vector work
                            gs = rp.tile([128, 2, 1], F32, tag="gs", name=f"gs{pair}")
                            nc.vector.reciprocal(gs, pu2[pair][:, :, C : C + 1])
                            gss.append(gs)
                        for pair in range(2):
                            gs = gss[pair]
                            for j in range(2):
                                hi = q4 * 4 + pair * 2 + j
                                if j == 0:
                                    # gpsimd can't read PSUM: scale on vector,
                                    # residual add on gpsimd (SBUF only)
                                    r2 = rp.tile([128, C], BF16, tag="r2")
                                    nc.vector.tensor_scalar_mul(
                                        r2, pu2[pair][:, j, :C], gs[:, j, :]
                                    )
                                    nc.gpsimd.tensor_add(
                                        ost[:, hi, :], r2, xrt[:, hi, :]
                                    )
                                else:
                                    nc.vector.scalar_tensor_tensor(
                                        ost[:, hi, :], pu2[pair][:, j, :C], gs[:, j, :],
                                        xrt[:, hi, :], op0=ALU.mult, op1=ALU.add,
                                    )
                    nc.sync.dma_start(out_w[:, hsl, :], ost[:])

    nc.compile()
    return nc


_NC_CACHE = None


def _get_nc():
    global _NC_CACHE
    if _NC_CACHE is None:
        _NC_CACHE = build_program()
    return _NC_CACHE


def make_in_maps(x, wq, bq, wk, bk, wv, bv):
    bf = ml_dtypes.bfloat16
    f8 = ml_dtypes.float8_e4m3
    x = np.asarray(x, np.float32)
    xres_full = (x + GAMMA * np.asarray(bv, np.float32)).astype(bf)
    # host-side transpose to channel-on-partition fp8: [b, 128, 2, HWPIX]
    xt8_full = np.ascontiguousarray(
        x.reshape(B, HWPIX, 2, 128).transpose(0, 3, 2, 1)
    ).astype(f8)
    wqk = np.concatenate(
        [np.asarray(wq, np.float32), np.asarray(wk, np.float32)], axis=1
    )  # [C, 64]
    wqk8 = np.ascontiguousarray(wqk.reshape(2, 128, 2 * D).transpose(1, 0, 2)).astype(f8)
    wv8 = np.ascontiguousarray(
        np.asarray(wv, np.float32).reshape(2, 128, C).transpose(1, 0, 2)
    ).astype(f8)
    eye = np.eye(128, dtype=bf)
    mask4 = np.tile((1.0 - np.eye(128, dtype=np.float32)).astype(bf), (1, 4))

    in_maps = []
    for ci in range(NCORES):
        sl = slice(ci * BPC, (ci + 1) * BPC)
        in_maps.append(
            {
                "xt8": xt8_full[sl],
                "xres": xres_full[sl].reshape(BPC, HWPIX, C),
                "wqk8": wqk8,
                "wv8": wv8,
                "bq_f": np.asarray(bq, np.float32).reshape(D, 1),
                "bk_f": np.asarray(bk, np.float32).reshape(D, 1),
                "eye_b": eye,
                "mask4_b": mask4,
            }
        )
    return in_maps


def kernel(x, wq, bq, wk, bk, wv, bv):
    in_maps = make_in_maps(x, wq, bq, wk, bk, wv, bv)
    nc = _get_nc()
    res = run_bass_kernel_spmd(nc, in_maps, core_ids=list(range(NCORES)))
    outs = [
        np.asarray(res.results[ci]["out"]).astype(np.float32).reshape(BPC, H, W, C)
        for ci in range(NCORES)
    ]
    return np.concatenate(outs, axis=0)
